# revision 2
# baseline (speedup 1.0000x reference)
"""Multi-head attention (B=16, S=512, H=768, NH=12) on 8 Trainium2 NeuronCores.

Strategy: data-parallel over batch - 2 batches per core, no collectives.

Dataflow (all matmul inputs bf16, fp32 PSUM accumulation). The kernel is
PE-bound (~95us of matmul streaming at 2.4GHz per core); structure notes:

  - warmup: ~6 junk matmuls on a memset tile run during the input-DMA
    preamble so the PE HAM clock-gate is already at 8/8 when the first
    real matmul issues (the gate needs ~3.4us of sustained PE activity).
  - inputs are host-packed for large DMA descriptor rows: x chunks carry
    both batches side by side ([128, 2S] per contraction chunk, 2KB rows),
    wv / wo are single [128, KC*H] tiles (9KB rows). wqk stays as 12
    per-output-block DMAs so blocks arrive at the per-ob consumption rate.
    All input DMA issue rides the sync + scalar HWDGE rings (scalar's ring
    is DMA-free during attention so EXP issue is never stuck behind
    descriptor generation).
  - QKV projection computed transposed (q^T/k^T with head dim on
    partitions); v in natural [s, o] orientation into per-head slots whose
    upper 64 columns are ones so the attention-value matmul also emits the
    softmax denominator. The v-projection bias folds into the output bias
    (y = sum p (v0+bv)/sum p = y0+bv).
  - scores^T = k^T.T @ q^T per head pair into one 2-bank PSUM tile; the
    pair's matmuls use disjoint PE row-groups and run concurrently; one
    wide exp per (pair, kb) on ScalarE with the 1/sqrt(dk) scale fused
    (no max-subtraction: |scores| < ~10 for these activations).
  - attention is ACT(exp)-bound per pair; PE idle inside attention(b0) is
    filled with batch 1's whole QKV projection, attention(b1) with batch
    0's output projection + the batch-1 drain's A passes.
  - batch 0's output projection bias is applied by the DMA engines: the
    output DRAM rows are prefilled with the (folded) bias early on the
    gpsimd queue, and batch-0 stores use SWDGE accum_op=add - no bias
    matmuls and no extra vector work. (Same-queue FIFO + the per-partition
    SDMA engine swizzle order the prefill before the accumulating store.)
  - attention(b1) hoists the LAST pair's scores+exps into the previous
    pair's slot, so the final exp (which gates the last AV group, the
    normalize chain, and the drain B passes) retires ~4us earlier and the
    PE never idles waiting for ScalarE at the end.
  - output stores are one packed [128, 768] f32 tile + one dma_start per
    (batch, s-block): 3KB descriptor rows, half the descriptor count.
    Batch-1 (drain) stores go on the two HWDGE rings in parallel with the
    bias applied via per-group PE bias matmuls (start of each accumulation
    group), since serializing accum stores on the single SWDGE queue would
    stretch the tail.

attn_mask from the reference setup is all-ones; a non-trivial mask falls
back to a numpy implementation.
"""

import sys

sys.path.insert(0, "/opt/trn_rl_repo")

import numpy as np

B, S, H, NH = 16, 512, 768, 12
DK = H // NH  # 64
N_CORES = 8
NB = B // N_CORES  # batches per core = 2
KC = H // 128  # 6 contraction chunks
SBLK = S // 128  # 4 s-blocks of 128
VW = 2 * DK  # 128: per-head v slot width (64 v cols + 64 ones cols)
KORD = [0, 2, 5, 1, 3, 4]  # contraction order matched to x-chunk DMA arrival

_PROG_CACHE = {}


def _build_program():
    import concourse.tile as tile
    from concourse import bacc, mybir

    f32 = mybir.dt.float32
    cdt = mybir.dt.bfloat16
    EXP = mybir.ActivationFunctionType.Exp
    ADD = mybir.AluOpType.add

    nc = bacc.Bacc("TRN2", target_bir_lowering=False, debug=False,
                   num_devices=N_CORES)

    xab_d = nc.declare_dram_parameter("xab", [KC, 128, NB * S], cdt, isOutput=False)
    wqk_d = nc.declare_dram_parameter("wqkb", [2 * KC, 128, KC * 128], cdt, isOutput=False)
    wv_d = nc.declare_dram_parameter("wvb", [128, KC * H], cdt, isOutput=False)
    wo_d = nc.declare_dram_parameter("wob", [128, KC * H], cdt, isOutput=False)
    bqk_d = nc.declare_dram_parameter("bqk", [128, 2 * KC], f32, isOutput=False)
    bo2_d = nc.declare_dram_parameter("bo2", [1, H], cdt, isOutput=False)
    bo2bc_d = nc.declare_dram_parameter("bo2bc", [128, H], f32, isOutput=False)
    out_d = nc.declare_dram_parameter("out", [NB, S, H], f32, isOutput=True)

    with tile.TileContext(nc) as tc:
        from contextlib import ExitStack

        with ExitStack() as ctx:
            ep = ctx.enter_context
            wqk_p = ep(tc.tile_pool(name="wqk", bufs=1))
            wv_p = ep(tc.tile_pool(name="wv", bufs=1))
            wo_p = ep(tc.tile_pool(name="wo", bufs=1))
            x_p = ep(tc.tile_pool(name="xp", bufs=1))
            qk_p = ep(tc.tile_pool(name="qk", bufs=2))
            v_p = ep(tc.tile_pool(name="vp", bufs=2))
            pt_p = ep(tc.tile_pool(name="pt", bufs=8))
            yb_p = ep(tc.tile_pool(name="yb", bufs=2))
            rc_p = ep(tc.tile_pool(name="rc", bufs=4))
            tm_p = ep(tc.tile_pool(name="tm", bufs=3))
            cb_p = ep(tc.tile_pool(name="cb", bufs=1))
            pj_ps = ep(tc.tile_pool(name="pj", bufs=2, space="PSUM"))
            sc_ps = ep(tc.tile_pool(name="sc", bufs=2, space="PSUM"))
            ya_ps = ep(tc.tile_pool(name="ya", bufs=2, space="PSUM"))

            sy, sc, gp = nc.sync, nc.scalar, nc.gpsimd

            # ---- PE warmup: junk matmuls on a memset tile keep the PE busy
            # through the input-DMA preamble so the HAM clock gate reaches
            # 8/8 before the first real matmul ----
            wu_t = cb_p.tile([128, 512], cdt, tag="wu", name="wu_t")
            nc.gpsimd.memset(wu_t[:], 0.125)
            wu_ps = pj_ps.tile([128, S], f32, tag="pj", name="pj_ps_t")
            for j in range(6):
                nc.tensor.matmul(
                    wu_ps[:],
                    lhsT=wu_t[:, 0:128],
                    rhs=wu_t[:],
                    start=(j == 0), stop=(j == 5),
                )

            # ---- constants ----
            on_t = cb_p.tile([1, 128], cdt, tag="ones", name="on_t")
            nc.gpsimd.memset(on_t[:], 1.0)
            bqk_t = cb_p.tile([128, 2 * KC], f32, tag="bqk", name="bqk_t")
            bo2b_t = cb_p.tile([1, H], cdt, tag="bo2b", name="bo2b_t")
            bo2bc_t = cb_p.tile([128, H], f32, tag="bo2bc", name="bo2bc_t")

            # ---- input DMA plan: two HWDGE FIFOs (sync, scalar) sequenced
            # against the projection loop's consumption deadlines; gpsimd
            # (SWDGE) carries the batch-0 bias prefill + accum stores ----
            x_t = [None] * KC

            def x_dma(k, q):
                t = x_p.tile([128, NB * S], cdt, tag=f"x{k}", name=f"x_{k}")
                q.dma_start(out=t[:], in_=xab_d.ap()[k])
                x_t[k] = t

            wqk_t = [None] * (2 * KC)

            def wqk_dma(ob, q):
                t = wqk_p.tile([128, KC * 128], cdt, tag=f"wqk{ob}", name=f"wqk{ob}")
                q.dma_start(out=t[:], in_=wqk_d.ap()[ob])
                wqk_t[ob] = t

            wv_t = wv_p.tile([128, KC * H], cdt, tag="wv", name="wv_t")
            wo_t = wo_p.tile([128, KC * H], cdt, tag="wo", name="wo_t")

            plan = [
                lambda: x_dma(0, sy),
                lambda: x_dma(2, sc),
                lambda: x_dma(5, sy),
                lambda: x_dma(1, sc),
                lambda: wqk_dma(0, sy),
                lambda: wqk_dma(1, sc),
                lambda: x_dma(4, sy),
                lambda: x_dma(3, sc),
                lambda: sc.dma_start(out=bqk_t[:], in_=bqk_d.ap()),
                lambda: wqk_dma(2, sy),
                lambda: wqk_dma(3, sc),
                lambda: wqk_dma(4, sy),
                lambda: wqk_dma(5, sc),
                lambda: wqk_dma(6, sy),
                lambda: wqk_dma(7, sc),
                lambda: wqk_dma(8, sy),
                lambda: wqk_dma(9, sc),
                lambda: wqk_dma(10, sy),
                lambda: wqk_dma(11, sc),
                lambda: sy.dma_start(out=wv_t[:], in_=wv_d.ap()),
                lambda: sc.dma_start(out=wo_t[:], in_=wo_d.ap()),
                lambda: sc.dma_start(out=bo2b_t[:], in_=bo2_d.ap()),
                # bias prefill: load the broadcast folded bias, then write it
                # into batch 0's output rows; batch-0 stores accumulate onto
                # it via SWDGE accum_op (same gpsimd FIFO -> ordered)
                lambda: gp.dma_start(out=bo2bc_t[:], in_=bo2bc_d.ap()),
            ]
            for emit in plan:
                emit()
            for sb in range(SBLK):
                gp.dma_start(out=out_d.ap()[0, 128 * sb:128 * (sb + 1), :],
                             in_=bo2bc_t[:])

            # ---- building blocks ----
            qk_store = {}

            def qk_chunk(b, ob):
                def emit():
                    ps = pj_ps.tile([128, S], f32, tag="pj", name="pj_ps_t")
                    for j, k in enumerate(KORD):
                        nc.tensor.matmul(
                            ps[:],
                            lhsT=wqk_t[ob][:, 128 * k:128 * (k + 1)],
                            rhs=x_t[k][:, S * b:S * (b + 1)],
                            start=(j == 0), stop=(j == KC - 1),
                        )
                    t = qk_p.tile([128, S], cdt, tag=f"qk{ob}", name=f"qk{b}_{ob}")
                    nc.vector.tensor_scalar_add(out=t[:], in0=ps[:],
                                                scalar1=bqk_t[:, ob:ob + 1])
                    qk_store[(b, ob)] = t
                return emit

            v_store = {}

            def v_chunk(b, sb, og):
                def emit():
                    if og == 0:
                        vt = v_p.tile([128, NH * VW], cdt, tag=f"v{sb}", name=f"v{b}_{sb}")
                        # upper 64 cols of each head slot must be 1.0 (the
                        # softmax-denominator columns); set the whole tile and
                        # let the copies below overwrite the v columns
                        nc.gpsimd.memset(vt[:], 1.0)
                        v_store[(b, sb)] = vt
                    vt = v_store[(b, sb)]
                    o0, w = (0, 512) if og == 0 else (512, 256)
                    ps = pj_ps.tile([128, S], f32, tag="pj", name="pj_ps_t")
                    for k in range(KC):
                        nc.tensor.matmul(
                            ps[:, :w],
                            lhsT=x_t[k][:, S * b + 128 * sb:S * b + 128 * (sb + 1)],
                            rhs=wv_t[:, H * k + o0:H * k + o0 + w],
                            start=(k == 0), stop=(k == KC - 1),
                        )
                    nh = w // DK
                    h0 = o0 // DK
                    src = ps[:, :w].rearrange("p (h c) -> p h c", h=nh)
                    dst = vt[:].rearrange("p (h c) -> p h c", h=NH)[:, h0:h0 + nh, 0:DK]
                    nc.vector.tensor_copy(out=dst, in_=src)
                return emit

            def fproj(b, sb, yb_list):
                """Batch-0 output projection for one s-block: A = heads 0-4
                for both column groups (no bias matmul - the DMA accumulates
                onto prefilled bias rows), B = head 5 + copies + one packed
                SWDGE accum store."""
                st = {}

                def emit_a():
                    p1 = pj_ps.tile([128, 512], f32, tag="pj", name="pj_ps_t")
                    p2 = pj_ps.tile([128, 512], f32, tag="pj", name="pj_ps_t")
                    st["ps"] = [p1[:, :], p2[:, 0:256]]
                    for (o0, w), ps in zip(((0, 512), (512, 256)), st["ps"]):
                        for hb in range(KC - 1):
                            nc.tensor.matmul(
                                ps[:, :w],
                                lhsT=yb_list[hb][:, 128 * sb:128 * (sb + 1)],
                                rhs=wo_t[:, H * hb + o0:H * hb + o0 + w],
                                start=(hb == 0), stop=False,
                            )

                def emit_b():
                    hb = KC - 1
                    for (o0, w), ps in zip(((0, 512), (512, 256)), st["ps"]):
                        nc.tensor.matmul(
                            ps[:, :w],
                            lhsT=yb_list[hb][:, 128 * sb:128 * (sb + 1)],
                            rhs=wo_t[:, H * hb + o0:H * hb + o0 + w],
                            start=False, stop=True,
                        )
                    ot = tm_p.tile([128, H], f32, tag="ot", name="ot")
                    nc.vector.tensor_copy(out=ot[:, 0:512], in_=st["ps"][0])
                    nc.vector.tensor_copy(out=ot[:, 512:768], in_=st["ps"][1])
                    gp.dma_start(
                        out=out_d.ap()[b, 128 * sb:128 * (sb + 1), :],
                        in_=ot[:],
                        accum_op=ADD,
                    )
                return emit_a, emit_b

            def attention(b, pending, late_pending=None, yb_out=None,
                          kb_pops=(1, 3), hoist_last=False):
                """Head-pair attention for batch b; pops `pending` PE-filler
                closures into the ACT-bound gaps. With hoist_last, the last
                pair's scores+exps are emitted during the previous pair's
                slot so the final exp retires well before the drain needs
                it. `late_pending` items may depend on every pair of this
                batch but the last, so they only pop after the last pair's
                AV (where they bridge the final normalize chain)."""
                late_pending = late_pending or []
                yb_t = [yb_p.tile([128, S], cdt, tag=f"yb{hb}", name=f"yb{b}_{hb}")
                        for hb in range(KC)]
                if yb_out is not None:
                    yb_out.extend(yb_t)
                n_pair = NH // 2
                pts_store = {}

                def scores_exp(hp, pop_kbs):
                    pair = (2 * hp, 2 * hp + 1)
                    q_tile = qk_store[(b, hp)]
                    k_tile = qk_store[(b, KC + hp)]
                    pts = {h: [] for h in pair}
                    for kb in range(SBLK):
                        scp = sc_ps.tile([128, 2 * S], f32, tag="sc", name="sc_ps_t")
                        for hi, h in enumerate(pair):
                            krow = (h % 2) * DK
                            nc.tensor.matmul(
                                scp[:, hi * S:(hi + 1) * S],
                                lhsT=k_tile[krow:krow + DK, 128 * kb:128 * (kb + 1)],
                                rhs=q_tile[krow:krow + DK, :],
                                start=True, stop=True,
                            )
                        ptt = pt_p.tile([128, 2 * S], cdt, tag="ptt", name="ptt")
                        nc.scalar.activation(out=ptt[:], in_=scp[:], func=EXP,
                                             scale=float(1.0 / np.sqrt(DK)))
                        for hi, h in enumerate(pair):
                            pts[h].append(ptt[:, hi * S:(hi + 1) * S])
                        if kb in pop_kbs and pending:
                            pending.pop(0)()
                    pts_store[hp] = pts

                def av_norm(hp, last):
                    pair = (2 * hp, 2 * hp + 1)
                    pts = pts_store.pop(hp)
                    yps = {h: ya_ps.tile([128, S], f32, tag="ya", name="ya_ps_t")
                           for h in pair}
                    for kb in range(SBLK):
                        if kb == SBLK - 1 and not last and pending:
                            # filler between the kb2 and kb3 AV groups hides
                            # the latency of the pair's last exp, which the
                            # kb3 AV matmuls would otherwise wait on
                            pending.pop(0)()
                        for h in pair:
                            nc.tensor.matmul(
                                yps[h][:],
                                lhsT=v_store[(b, kb)][:, VW * h:VW * (h + 1)],
                                rhs=pts[h][kb][:],
                                start=(kb == 0), stop=(kb == SBLK - 1),
                            )
                    if last:
                        # the drain A passes run after the last AV so they
                        # bridge this pair's normalize chain
                        while late_pending:
                            late_pending.pop(0)()
                    # PSUM rows 64..127 of each head's AV tile hold the
                    # softmax denominator broadcast across 64 partitions
                    den = rc_p.tile([128, S], f32, tag="rec0", name="den")
                    for hi, h in enumerate(pair):
                        nc.vector.tensor_copy(out=den[hi * DK:(hi + 1) * DK, :],
                                              in_=yps[h][DK:2 * DK, :])
                    rec = rc_p.tile([128, S], f32, tag="rec1", name="rec")
                    nc.vector.reciprocal_approx_fast(out=rec[:], in_=den[:])
                    for hi, h in enumerate(pair):
                        krow = hi * DK
                        if last and b == 1:
                            nc.vector.tensor_mul(
                                out=yb_t[hp][krow:krow + DK, 0:256],
                                in0=yps[h][0:DK, 0:256],
                                in1=rec[krow:krow + DK, 0:256])
                        else:
                            nc.vector.tensor_mul(out=yb_t[hp][krow:krow + DK, :],
                                                 in0=yps[h][0:DK, :],
                                                 in1=rec[krow:krow + DK, :])
                    if last and b == 1:
                        for hi, h in enumerate(pair):
                            krow = hi * DK
                            nc.vector.tensor_mul(
                                out=yb_t[hp][krow:krow + DK, 256:512],
                                in0=yps[h][0:DK, 256:512],
                                in1=rec[krow:krow + DK, 256:512])

                for hp in range(n_pair):
                    last = hp == n_pair - 1
                    if hoist_last and last:
                        # scores+exps already emitted in the previous slot
                        av_norm(hp, True)
                        continue
                    scores_exp(hp, kb_pops)
                    if hoist_last and hp == n_pair - 2:
                        scores_exp(n_pair - 1, kb_pops)
                    av_norm(hp, last and not hoist_last)
                while pending:
                    pending.pop(0)()
                while late_pending:
                    late_pending.pop(0)()
                return yb_t

            # ---- batch 0: QKV projection (DMA-paced head phase) ----
            for ob in range(2 * KC):
                qk_chunk(0, ob)()
            for sb in range(SBLK):
                for og in range(2):
                    v_chunk(0, sb, og)()

            # ---- attention(0), filled with QKV(1); defer batch 1's last
            # head-pair q/k blocks into attention(1) for ACT/PE balance ----
            pend0 = []
            for ob in range(2 * KC):
                if ob in (KC - 1, 2 * KC - 1):
                    continue
                pend0.append(qk_chunk(1, ob))
            for sb in range(SBLK):
                for og in range(2):
                    pend0.append(v_chunk(1, sb, og))
            yb0 = attention(0, pend0)

            # ---- attention(1), filled with deferred q/k blocks + fproj(0);
            # the first drain chunks' A passes ride along at the end so the
            # PE stays busy through the last pair's normalize chain ----
            yb1_holder = []
            drain = []

            def drain_fproj(sb, act_copy, pool, eng):
                """Batch-1 output projection for one s-block. Bias comes from
                PE bias matmuls (accumulation-group starts); stores are plain
                HWDGE on the sync/scalar rings (parallel, out of the single
                SWDGE queue's way)."""
                st = {}

                def group(ps, o0, w, phase):
                    if phase == "a":
                        nc.tensor.matmul(ps[:, :w], lhsT=on_t[:],
                                         rhs=bo2b_t[:, o0:o0 + w],
                                         start=True, stop=False)
                        for hb in range(KC - 1):
                            nc.tensor.matmul(
                                ps[:, :w],
                                lhsT=yb1_holder[hb][:, 128 * sb:128 * (sb + 1)],
                                rhs=wo_t[:, H * hb + o0:H * hb + o0 + w],
                                start=False, stop=False)
                    else:
                        hb = KC - 1
                        nc.tensor.matmul(
                            ps[:, :w],
                            lhsT=yb1_holder[hb][:, 128 * sb:128 * (sb + 1)],
                            rhs=wo_t[:, H * hb + o0:H * hb + o0 + w],
                            start=False, stop=True)

                def emit_a():
                    if pool == "sc":
                        ps = sc_ps.tile([128, 2 * S], f32, tag="sc", name="sc_ps_t")
                        st["ps"] = [ps[:, 0:512], ps[:, 512:768]]
                    else:
                        p1 = pj_ps.tile([128, 512], f32, tag="pj", name="pj_ps_t")
                        p2 = pj_ps.tile([128, 512], f32, tag="pj", name="pj_ps_t")
                        st["ps"] = [p1[:, :], p2[:, 0:256]]
                    for (o0, w), ps in zip(((0, 512), (512, 256)), st["ps"]):
                        group(ps, o0, w, "a")

                def emit_b():
                    for (o0, w), ps in zip(((0, 512), (512, 256)), st["ps"]):
                        group(ps, o0, w, "b")
                    ot = tm_p.tile([128, H], f32, tag="ot", name="ot")
                    if act_copy:
                        nc.scalar.copy(out=ot[:, 0:512], in_=st["ps"][0])
                        nc.vector.tensor_copy(out=ot[:, 512:768], in_=st["ps"][1])
                    else:
                        nc.vector.tensor_copy(out=ot[:, 0:512], in_=st["ps"][0])
                        nc.scalar.copy(out=ot[:, 512:768], in_=st["ps"][1])
                    eng.dma_start(
                        out=out_d.ap()[1, 128 * sb:128 * (sb + 1), :],
                        in_=ot[:],
                    )
                return emit_a, emit_b

            for sb in range(SBLK):
                drain.append(drain_fproj(sb, act_copy=True,
                                         pool=("pj" if sb == 2 else "sc"),
                                         eng=[sy, sc, sy, sc][sb]))

            pend1 = [qk_chunk(1, KC - 1), qk_chunk(1, 2 * KC - 1)]
            for sb in range(SBLK):
                ea, eb = fproj(0, sb, yb0)
                pend1.append(ea)
                pend1.append(eb)
            attention(1, pend1,
                      late_pending=[drain[2][0], drain[0][0], drain[1][0]],
                      yb_out=yb1_holder, kb_pops=(3,), hoist_last=True)

            # ---- fproj(1) drain: B passes (head 5 + packed store)
            # interleaved with the remaining A pass ----
            drain[0][1]()
            drain[3][0]()
            drain[1][1]()
            drain[2][1]()
            drain[3][1]()

    nc.compile()
    return nc


def get_program():
    if "nc" not in _PROG_CACHE:
        _PROG_CACHE["nc"] = _build_program()
    return _PROG_CACHE["nc"]


def make_in_maps(x, w_qkv_w, w_qkv_b, w_o_w, w_o_b):
    import ml_dtypes
    np_cdt = ml_dtypes.bfloat16
    x = np.asarray(x, np.float32)
    xT = np.transpose(x, (0, 2, 1)).astype(np_cdt)  # [B, H, S]
    wqkvT = np.asarray(w_qkv_w, np.float32).T  # [H, 3H]
    # q,k columns blocked per transposed output block:
    # wqkb[ob][p][k*128+c] = wqkvT[k*128+p, ob*128+c]
    t = wqkvT[:, :2 * H].reshape(KC, 128, 2 * KC, 128)
    wqkb = np.ascontiguousarray(t.transpose(2, 1, 0, 3).reshape(2 * KC, 128, KC * 128).astype(np_cdt))
    # v columns in contraction-chunk-major rows: wvb[p][k*H+c] = wqkvT[k*128+p, 2H+c]
    tv = wqkvT[:, 2 * H:].reshape(KC, 128, H)
    wvb = np.ascontiguousarray(tv.transpose(1, 0, 2).reshape(128, KC * H).astype(np_cdt))
    # w_o^T in contraction-chunk-major rows: wob[p][hb*H+c] = w_o^T[hb*128+p, c]
    woT = np.asarray(w_o_w, np.float32).T  # [H, H]
    wob = np.ascontiguousarray(
        woT.reshape(KC, 128, H).transpose(1, 0, 2).reshape(128, KC * H).astype(np_cdt))
    # qk bias as [128, 12] f32: bqk[p, j] = w_qkv_b[j*128+p]
    bqk = np.ascontiguousarray(
        np.asarray(w_qkv_b, np.float32)[:2 * H].reshape(2 * KC, 128).T)
    # v-projection bias folded into the output bias (y = Sum p (v0+bv)/Sum p
    # = y0 + bv, so out = y0 @ w_o^T + (bo + w_o @ bv))
    bv_f = np.asarray(w_qkv_b, np.float32)[2 * H:]
    bo2_row = np.asarray(w_o_b, np.float32) + np.asarray(w_o_w, np.float32) @ bv_f
    bo2 = np.ascontiguousarray(bo2_row.reshape(1, H).astype(np_cdt))
    bo2bc = np.ascontiguousarray(
        np.broadcast_to(bo2_row.astype(np.float32), (128, H)))
    return [
        {
            # x chunks packed [KC, 128, NB*S]: both batches side by side so
            # DMA descriptor rows are 2KB
            "xab": np.ascontiguousarray(
                np.stack([xT[NB * c:NB * (c + 1)]
                          .reshape(NB, KC, 128, S)[:, k]  # [NB, 128, S]
                          .transpose(1, 0, 2).reshape(128, NB * S)
                          for k in range(KC)])),
            "wqkb": wqkb,
            "wvb": wvb,
            "wob": wob,
            "bqk": bqk,
            "bo2": bo2,
            "bo2bc": bo2bc,
        }
        for c in range(N_CORES)
    ]


def _numpy_fallback(x, attn_mask, w_qkv_w, w_qkv_b, w_o_w, w_o_b):
    x = np.asarray(x, np.float64)
    qkv = x @ np.asarray(w_qkv_w, np.float64).T + np.asarray(w_qkv_b, np.float64)
    q, k, v = np.split(qkv, 3, axis=-1)

    def heads(t):
        return t.reshape(B, S, NH, DK).transpose(0, 2, 1, 3)

    q, k, v = heads(q), heads(k), heads(v)
    s = np.einsum("bhqd,bhkd->bhqk", q, k) / np.sqrt(DK)
    mask = np.asarray(attn_mask, bool)[:, None, None, :]
    s = np.where(mask, s, -np.inf)
    s = s - s.max(axis=-1, keepdims=True)
    p = np.exp(s)
    p = p / p.sum(axis=-1, keepdims=True)
    y = np.einsum("bhqk,bhkd->bhqd", p, v)
    y = y.transpose(0, 2, 1, 3).reshape(B, S, H)
    out = y @ np.asarray(w_o_w, np.float64).T + np.asarray(w_o_b, np.float64)
    return out.astype(np.float32)


def kernel(x, attn_mask, w_qkv_w, w_qkv_b, w_o_w, w_o_b):
    if not bool(np.all(np.asarray(attn_mask))):
        return _numpy_fallback(x, attn_mask, w_qkv_w, w_qkv_b, w_o_w, w_o_b)

    from concourse.bass_utils import run_bass_kernel_spmd

    nc = get_program()
    in_maps = make_in_maps(x, w_qkv_w, w_qkv_b, w_o_w, w_o_b)
    res = run_bass_kernel_spmd(nc, in_maps, list(range(N_CORES)))
    out = np.concatenate([res.results[c]["out"] for c in range(N_CORES)], axis=0)
    return out.astype(np.float32)
